# revision 1
# baseline (speedup 1.0000x reference)
"""Trainium2 Bass kernel for nn_AttentionLayer (GAT-style masked attention).

Computes, for full inputs:
    h1 = emb_src @ W                      [8000, 128]
    g  = emb_dest @ (W @ W2)              [10000, 128]
    e  = g @ h1.T                         [10000, 8000]
    s  = lrelu(e, 0.2) * (1/sqrt(128))    masked to -inf where bias <= 0
    att = softmax(s, axis=1)
    out = att @ ft                        [10000, 1]   (ft = nan-cleaned feature_src)

Sharding: N_dest split across 8 NeuronCores (1250 rows each); emb_src /
feature_src / W / W2 replicated. No collectives. Softmax is unnormalized
(numer/denom) — no max subtraction needed since |scale*lrelu(e)| <= ~15.

Layout: TRANSPOSED on-device — scores are computed as e.T tiles
[src=partition, dest=free] so that BOTH softmax reductions (denominator
sum(u) and numerator sum(u*ft)) run on the Tensor engine as accumulating
matmuls with lhsT = [ones | ft_chunk], leaving DVE/ACT/Pool only the
mask + LeakyReLU + exp elementwise chain. Host-side staging is layout
only: bias is staged transposed+tile-contiguous [63,128,1280] (one
contiguous 640KB DMA per src tile), embeddings transposed, ft staged
partition-major [128,63].

Per-core device pipeline, per src tile s (63 tiles):
    DMA:   btT    = bias.T tile              [128,1250] f32 (contiguous)
    ACT:   amask  = Relu(-1e30 * btT)        {0 keep, +inf masked} fp16
                                             (emitted one tile EARLY so the
                                             in-order ACT queue never stalls
                                             the next mask behind this exp)
    PE:    psE    = SCALE * h1T_s.T @ gts    f32 PSUM (3 bank chunks)
    DVE:   sm     = psE - amask              (scalar_tensor_tensor, fp16)
    DVE:   y      = 0.2 * sm                 (4x mode)
    DVE:   t      = max(sm, y)               (2x; = lrelu(scale*e), -inf masked)
    ACT:   u      = Exp(t) -> bf16           (exp(-inf) == 0)
    PE:    psR   += [ones | ft_s].T @ u      (emitted one tile LATE so the
                                             in-order PE queue never stalls
                                             the next e-matmul behind it)
Final: out = psR[num] / psR[den] per dest column, three row DMAs.

Steady state is ACT+DVE throughput-bound (~2.7us/tile); bias DMAs own the
sync queue (first 10 tiles interleaved ahead of the emb_srcT preamble
stream) so neither mask generation nor h1T production starves.
"""
import os
import sys

sys.path.insert(0, "/opt/trn_rl_repo")

import numpy as np

_CACHE = {}

N_DEST, N_SRC, IN_DIM, HID = 10000, 8000, 256, 128
N_CORES = 8
ND = N_DEST // N_CORES            # 1250 dest rows per core
NDP = 1250                        # dest width (free axis of transposed tiles)
NSP = 8064                        # src padded to 63 full 128-row tiles
NST = NSP // 128                  # 63 src tiles
SCALE = float(1.0 / np.sqrt(np.float32(HID)))

HC = 1000                         # h1T build chunk width
N_HC = N_SRC // HC                # 8

CHK = [(0, 512), (512, 512), (1024, NDP - 1024)]   # dest chunks (PSUM banks)
MGA = NDP                         # maskgen all on ACT
MBIG = 30000.0                    # mask magnitude (fp16-safe)


def _build_nc():
    import concourse.bass as bass
    import concourse.tile as tile
    from concourse import bacc, mybir
    from concourse.masks import make_identity
    from contextlib import ExitStack

    F32 = mybir.dt.float32
    F32R = mybir.dt.float32r
    BF16 = mybir.dt.bfloat16
    FP16 = mybir.dt.float16
    AF = mybir.ActivationFunctionType
    OP = mybir.AluOpType

    nc = bacc.Bacc("TRN2", target_bir_lowering=False, debug=False,
                   num_devices=N_CORES)

    bias_t = nc.declare_dram_parameter("biasT", [NST, 128, NDP], F32,
                                       isOutput=False)
    destT_t = nc.declare_dram_parameter("emb_destT", [IN_DIM, ND], F32,
                                        isOutput=False)
    srcT_t = nc.declare_dram_parameter("emb_srcT", [IN_DIM, N_SRC], F32,
                                       isOutput=False)
    ftc_t = nc.declare_dram_parameter("ft_cols", [128, NST], F32,
                                      isOutput=False)
    w_t = nc.declare_dram_parameter("W", [IN_DIM, HID], F32, isOutput=False)
    wt_t = nc.declare_dram_parameter("WT", [HID, IN_DIM], F32, isOutput=False)
    w2_t = nc.declare_dram_parameter("W2", [HID, HID], F32, isOutput=False)
    out_t = nc.declare_dram_parameter("out", [1, ND], F32, isOutput=True)

    with tile.TileContext(nc) as tc, ExitStack() as ctx:
        persist = ctx.enter_context(tc.tile_pool(name="persist", bufs=1))

        # persistent tiles
        gts = persist.tile([128, NDP], BF16)      # SCALE * g.T  [hid, dest]
        h1t = persist.tile([128, NSP], BF16)      # h1.T         [hid, src]
        ftw = persist.tile([128, 2 * NST], BF16)  # per-src-tile [ones | ft]

        # ================= main loop (pools concurrent with preamble:
        # no close barrier; main tiles start as soon as deps are ready)
        pbias = ctx.enter_context(tc.tile_pool(name="mn_bias", bufs=6))
        pmask = ctx.enter_context(tc.tile_pool(name="mn_mask", bufs=3))
        pt = ctx.enter_context(tc.tile_pool(name="mn_t", bufs=3))
        pu = ctx.enter_context(tc.tile_pool(name="mn_u", bufs=3))
        psm = ctx.enter_context(tc.tile_pool(name="mn_small", bufs=1))
        pacc = ctx.enter_context(
            tc.tile_pool(name="mn_acc", bufs=1, space="PSUM"))
        mps = ctx.enter_context(
            tc.tile_pool(name="mn_ps", bufs=2, space="PSUM"))
        pre = ctx.enter_context(tc.tile_pool(name="pre_sb", bufs=2))
        preb = ctx.enter_context(tc.tile_pool(name="pre_big", bufs=2))
        pps = ctx.enter_context(
            tc.tile_pool(name="pre_ps", bufs=1, space="PSUM"))

        # ---- ftw: per src tile s, columns [2s, 2s+1] = [ones, ft_s]
        ftc_sb = pre.tile([128, NST], F32, tag="ftc_sb")
        nc.sync.dma_start(out=ftc_sb, in_=ftc_t[:, :])
        ftw_v = ftw[:, :].rearrange("p (s two) -> p s two", two=2)
        nc.gpsimd.memset(ftw_v[:, :, 0], 1.0)
        nc.vector.tensor_copy(out=ftw_v[:, :, 1], in_=ftc_sb)


        # ---- W chunks ([K=in_sub, M=hid]) + bf16
        w_sb = pre.tile([128, 2, HID], F32R, tag="w_sb")
        for c in range(2):
            nc.sync.dma_start(out=w_sb[:, c, :],
                              in_=w_t[128 * c:128 * (c + 1), :].bitcast(F32R))
        w2_sb = pre.tile([128, HID], F32, tag="w2_sb")
        nc.sync.dma_start(out=w2_sb, in_=w2_t[:, :])

        # ---- Wc = W @ W2 as [K=in_sub, M=hid] chunks (lhsT = staged W.T)
        wt_sb = pre.tile([128, IN_DIM], F32, tag="wt_sb")
        nc.sync.dma_start(out=wt_sb, in_=wt_t[:, :])
        wc_sb = pre.tile([128, 2, HID], F32R, tag="wc_sb")
        for c in range(2):
            ps_mm = pps.tile([128, 512], F32, tag="ps_b")
            nc.tensor.matmul(ps_mm[:, :HID],
                             wt_sb[:, 128 * c:128 * (c + 1)], w2_sb,
                             start=True, stop=True)
            nc.scalar.copy(out=wc_sb[:, c, :], in_=ps_mm[:, :HID])

        # ---- bias tile DMA issue (sync queue, 6-buffer ring)
        bt_tiles = {}

        def issue_bt(s):
            btT = pbias.tile([128, NDP], F32, tag="btT", name=f"bt{s % 6}")
            nc.sync.dma_start(out=btT, in_=bias_t[s])
            bt_tiles[s] = btT

        issue_bt(0)
        issue_bt(1)

        # ---- emb_destT -> bf16 -> gts (= SCALE * Wc.T @ emb_dest.T)
        dsb = preb.tile([128, 2, ND], F32R, tag="dsb")
        for c in range(2):
            nc.sync.dma_start(out=dsb[:, c, :],
                              in_=destT_t[128 * c:128 * (c + 1), :].bitcast(F32R))
        for d0 in range(0, NDP, 512):
            dw = min(512, NDP - d0)
            ps_g = pps.tile([128, 512], F32, tag="ps_b")
            for c in range(2):
                nc.tensor.matmul(ps_g[:, :dw],
                                 wc_sb[:, c, :],
                                 dsb[:, c, d0:d0 + dw],
                                 start=(c == 0), stop=(c == 1))
            nc.scalar.activation(out=gts[:, d0:d0 + dw], in_=ps_g[:, :dw],
                                 func=AF.Copy, scale=SCALE)

        # ---- emb_srcT -> bf16 -> h1T (= W.T @ emb_src.T).
        # Only chunks 0-1 are produced in the preamble; chunks 2-7 are
        # injected into the main loop right before first use so the
        # in-order PE queue doesn't hold e-mm(0) behind 32 h1t matmuls
        # (whose last DMA dependency is the end of the 8MB src stream).
        nc.gpsimd.memset(h1t[:, N_SRC:NSP], 0.0)

        def emit_h1_chunk(j, eng):
            j0 = j * HC
            ssb = preb.tile([128, 2, HC], F32R, tag="ssb",
                            name=f"ssb{j % 2}")
            for c in range(2):
                eng.dma_start(
                    out=ssb[:, c, :],
                    in_=srcT_t[128 * c:128 * (c + 1),
                               j0:j0 + HC].bitcast(F32R))
            for half in range(2):
                ps_h = pps.tile([128, 512], F32, tag="ps_b",
                                name=f"psh{j % 2}{half}")
                for c in range(2):
                    nc.tensor.matmul(
                        ps_h[:, :500], w_sb[:, c, :],
                        ssb[:, c, half * 500:half * 500 + 500],
                        start=(c == 0), stop=(c == 1))
                if half == 0:
                    nc.scalar.copy(out=h1t[:, j0:j0 + 500],
                                   in_=ps_h[:, :500])
                else:
                    nc.vector.tensor_copy(out=h1t[:, j0 + 500:j0 + HC],
                                          in_=ps_h[:, :500])

        for j in range(2):
            emit_h1_chunk(j, nc.sync)
        for s in range(2, 6):
            issue_bt(s)

        psR = pacc.tile([128, 512], F32)  # rows 32k: denom, 32k+1: numer

        def mk_amask(s):
            # amask(s) emitted one tile EARLY so ACT's in-order queue never
            # stalls relu(s+1) behind exp(s) (which waits on DVE)
            btT = bt_tiles.pop(s)
            amask = pmask.tile([128, NDP], FP16, tag="amask",
                               name=f"am{s % 3}")
            nc.scalar.activation(out=amask, in_=btT, func=AF.Relu,
                                 scale=-1e30)
            return amask

        H1_TRIG = {4: 2, 12: 3, 20: 4, 28: 5, 36: 6, 44: 7}
        am_next = mk_amask(0)
        for s in range(NST):
            amask = am_next
            if s in H1_TRIG:
                emit_h1_chunk(H1_TRIG[s], nc.gpsimd)
            if s + 6 < NST:
                issue_bt(s + 6)
            if s + 1 < NST:
                am_next = mk_amask(s + 1)

            psE = mps.tile([128, 1536], F32, tag="psE")
            for (o, w) in CHK:
                nc.tensor.matmul(psE[:, o:o + w],
                                 h1t[:, 128 * s:128 * (s + 1)],
                                 gts[:, o:o + w], start=True, stop=True)

            sm = pt.tile([128, NDP], FP16, tag="sm")
            nc.vector.scalar_tensor_tensor(
                out=sm, in0=psE[:, :NDP], scalar=1.0, in1=amask,
                op0=OP.mult, op1=OP.subtract)
            y = pt.tile([128, NDP], FP16, tag="y")
            nc.vector.tensor_scalar_mul(y, sm, 0.2)
            t = pt.tile([128, NDP], FP16, tag="t")
            nc.vector.tensor_max(t, sm, y)
            u = pu.tile([128, NDP], BF16, tag="u")
            nc.scalar.activation(out=u, in_=t, func=AF.Exp)

            # reduce-mm for the PREVIOUS tile: keeps the in-order PE queue
            # from stalling e-mm(s+1) behind a reduce that waits on exp(s)
            if s > 0:
                up, sp = u_prev
                for k, (o, w) in enumerate(CHK):
                    nc.tensor.matmul(psR[32 * k:32 * k + 2, :w],
                                     ftw[:, 2 * sp:2 * sp + 2],
                                     up[:, o:o + w],
                                     start=(sp == 0), stop=False)
            u_prev = (u, s)

        up, sp = u_prev
        for k, (o, w) in enumerate(CHK):
            nc.tensor.matmul(psR[32 * k:32 * k + 2, :w],
                             ftw[:, 2 * sp:2 * sp + 2], up[:, o:o + w],
                             start=False, stop=True)

        # ---- finals: out = numer / denom on 3 partitions, 3 row DMAs
        rsb = psm.tile([66, 512], F32, tag="rsb")
        nc.scalar.copy(out=rsb, in_=psR[:66, :])
        d3 = psm.tile([3, 512], F32, tag="d3")
        n3 = psm.tile([3, 512], F32, tag="n3")
        for k in range(3):
            nc.sync.dma_start(out=d3[k:k + 1, :],
                              in_=rsb[32 * k:32 * k + 1, :])
            nc.sync.dma_start(out=n3[k:k + 1, :],
                              in_=rsb[32 * k + 1:32 * k + 2, :])
        rec3 = psm.tile([3, 512], F32, tag="rec3")
        nc.vector.reciprocal(out=rec3, in_=d3)
        o3 = psm.tile([3, 512], F32, tag="o3")
        nc.vector.tensor_mul(o3, n3, rec3)
        for k, (o, w) in enumerate(CHK):
            we = min(o + w, ND) - o
            nc.sync.dma_start(out=out_t[:, o:o + we], in_=o3[k:k + 1, :we])

    nc.compile()
    return nc


def _get_nc():
    if "nc" not in _CACHE:
        _CACHE["nc"] = _build_nc()
    return _CACHE["nc"]


def kernel(bias, emb_dest, emb_src, feature_src, W, W2, _trace=False):
    from concourse.bass_utils import run_bass_kernel_spmd

    bias = np.ascontiguousarray(bias, dtype=np.float32)
    emb_dest = np.ascontiguousarray(emb_dest, dtype=np.float32)
    emb_src = np.ascontiguousarray(emb_src, dtype=np.float32)
    ft = np.ascontiguousarray(feature_src, dtype=np.float32).reshape(-1)
    W = np.ascontiguousarray(W, dtype=np.float32)
    W2 = np.ascontiguousarray(W2, dtype=np.float32)

    nan_ind = np.isnan(ft)
    if nan_ind.any():
        # NaN source features: zero the feature and mask out the column
        # (matches reference semantics). Never hit for randn inputs.
        ft = np.where(nan_ind, 0.0, ft)
        bias = np.where(nan_ind.reshape(1, -1), -1.0, bias)

    srcT = np.ascontiguousarray(emb_src.T)          # [256, 8000]
    ftp = np.zeros(NSP, dtype=np.float32)
    ftp[:N_SRC] = ft
    ft_cols = np.ascontiguousarray(ftp.reshape(NST, 128).T)  # [128, 63]

    nc = _get_nc()
    in_maps = []
    for i in range(N_CORES):
        r0 = i * ND
        slabT = np.zeros((NSP, NDP), dtype=np.float32)
        slabT[:N_SRC, :ND] = bias[r0:r0 + ND].T
        in_maps.append({
            "biasT": slabT.reshape(NST, 128, NDP),
            "emb_destT": np.ascontiguousarray(emb_dest[r0:r0 + ND].T),
            "emb_srcT": srcT,
            "ft_cols": ft_cols,
            "W": W,
            "WT": np.ascontiguousarray(W.T),
            "W2": W2,
        })
    res = run_bass_kernel_spmd(nc, in_maps, list(range(N_CORES)),
                               trace=_trace)
    out = np.concatenate(
        [res.results[i]["out"].reshape(ND, 1) for i in range(N_CORES)], axis=0)
    if _trace:
        return out, res
    return out

